# revision 39
# baseline (speedup 1.0000x reference)
"""LoRA kernel for TRN2: y = (x @ A) @ B * scale, data-parallel over 8 cores.

Reference materializes W = (A@B)*scale [4096,4096] then x@W (~275 GFLOP).
Mathematically identical low-rank evaluation: u = x@(A*scale) [rows,8],
y = u@B — ~2 GFLOP, I/O + PE-streaming bound.

Per-core plan (rows sharded 8192/8 = 1024 rows/core, A/B replicated).

All matmul operands are bfloat16 (host converts; scale 2.0 is exact in
bf16): halves input DMA bytes vs f32 and keeps the PE at 1 col/cycle.
PSUM accumulation stays f32; y is drained PSUM->SBUF as bf16 and DMA'd
out as bf16 (host upcasts after the gather). Measured rel err ~5e-3
against the 2e-2 gate.

Cost-model facts this schedule is built around (CoreSim == grading
clock):
  - PE floor: (x elems + y elems)/128 = 65536 cyc @2.4GHz = 27.3us.
    P-state needs 3us of continuous PE busy — memset-fed warmups.
  - Every DMA->consumer edge costs ~2.2us fixed (HWDGE 630 + DGE 650 +
    sem 900); transfers run at 360 B/ns per queue, queues concurrent.
  - HWDGE (SP/ACT DMA) is a shared serializer ~630ns/instr; Pool DMAs
    (SWDGE) instead cost ~1us of Pool ENGINE each: keep instruction
    counts low, transfers big.
  - Tile deps are tile-granular: per-quarter x tiles, per-out-chunk y
    tiles.
  - Drains (PSUM [128,1024] f32 -> SBUF bf16): ACT 1038 / DVE 1192 ns.
    Interleaving y tiles into the NEXT block's u-matmuls on the PE
    spreads drain demand so the 2 drain engines never rate-lock the PE.

Row blocks [128, 256, 256, 256, 128]: small first block starts the PE
~0.8us earlier; small last block shortens the bare final y-phase tail.

Engine assignment:
  SP   : block-0 input quarters + half of remaining input + out DMAs
  Pool : other input quarters + out DMAs (SWDGE, private dead time)
  ACT  : A/B consts at t=0, half the y drains, a few out DMAs
  DVE  : memset warmup src, ut copies, other half of drains
  PE   : warmups; u(0); then u(b) interleaved with y(b-1); final y last

PSUM: ut double-buffer (2KB) + 3 x [128,1024] f32 y tiles (12KB)
= 14KB of 16KB.
"""

import os

import numpy as np
import ml_dtypes

os.environ.setdefault("MYCRO_LOCAL_CACHE", "1")

import concourse.bacc as bacc
import concourse.mybir as mybir
import concourse.tile as tile
from concourse.bass_utils import run_bass_kernel_spmd

F32 = mybir.dt.float32
BF16 = mybir.dt.bfloat16
BF16_NP = ml_dtypes.bfloat16

N_CORES = 8
BATCH, SEQ, D = 4, 2048, 4096
RANK = 8
SCALE = 16 / 8
ROWS = BATCH * SEQ            # 8192
R_CORE = ROWS // N_CORES      # 1024 rows per core
P = 128                       # partitions
KC = D // P                   # 32 feature chunks
NQ = 4                        # x-input quarters per block
QK = KC // NQ                 # 8 kc per quarter
BLOCKS = [128, 256, 256, 256, 128]
assert sum(BLOCKS) == R_CORE

_NC_CACHE = {}


def build(warmup=6, lead=8, lead_last=20, ysb_bufs=5, split=536,
          drain_mode="alt", ut_eng="scalar", memset=True,
          a_eng="gpsimd", b_eng="scalar"):
    nc = bacc.Bacc("TRN2", target_bir_lowering=False, debug=False)

    # Host packs x per-core as [P, R_CORE*KC] bf16 with
    # x row-block rb at col offset off*KC:  [p, (off+r)*... ] — see
    # _prep_in_maps: for each block, layout [P, KC*blk] with
    # xt[p, kc*blk + r] = x_shard[row0 + r, kc*128 + p].
    xt_d = nc.dram_tensor("xt", [P, R_CORE * KC], BF16, kind="ExternalInput")
    a_d = nc.dram_tensor("A", [P, KC * RANK], BF16, kind="ExternalInput")
    b_d = nc.dram_tensor("B", [RANK, D], BF16, kind="ExternalInput")
    y_d = nc.dram_tensor("y", [R_CORE, D], BF16, kind="ExternalOutput")

    with tile.TileContext(nc) as tc:
        with (
            tc.tile_pool(name="const", bufs=1) as cpool,
            tc.tile_pool(name="xq", bufs=8) as xqp,
            tc.tile_pool(name="usb", bufs=2) as usb,
            tc.tile_pool(name="ysb", bufs=ysb_bufs) as ysb,
            tc.tile_pool(name="ps_u", bufs=2, space="PSUM") as ps_u,
            tc.tile_pool(name="ps_y", bufs=3, space="PSUM") as ps_y,
        ):
            # A gates the very first u-matmul: issue it at t=0.
            engs = {"sync": nc.sync, "gpsimd": nc.gpsimd, "scalar": nc.scalar}
            a_sb = cpool.tile([P, KC, RANK], BF16)
            engs[a_eng].dma_start(
                a_sb[:], a_d[:, :].rearrange("p (kc r) -> p kc r", kc=KC)
            )
            # B in 4 chunks (a single [8,4096] DMA costs ~3.2us in the model).
            b_sb = cpool.tile([RANK, D], BF16)
            for i in range(4):
                engs[b_eng].dma_start(b_sb[:, i * 1024:(i + 1) * 1024],
                                      b_d[:, i * 1024:(i + 1) * 1024])

            # Warmups ramp the PE p-state while input DMA streams; fed by a
            # small DVE memset so they don't wait on any DMA. Short 128-col
            # matmuls: pe_busy_start lands earliest, count bridges the gap
            # until real input arrives.
            w_src = cpool.tile([P, P], BF16)
            if warmup and memset:
                nc.vector.memset(w_src[:], 0.0)
            for w in range(warmup):
                w_ps = ps_y.tile([P, 1024], F32, tag="y_ps", name=f"w{w}")
                nc.tensor.matmul(w_ps[:, :P], w_src[:], w_src[:])

            # Out-DMAs NEVER go on ACT/DVE: a DMA whose sem-wait is pending
            # holds that engine's SEQ, stalling the drain queue behind it.
            in_cycle = [nc.sync, nc.gpsimd]
            out_cycle = [nc.gpsimd, nc.sync]
            ii = oi = 0

            def in_eng():
                nonlocal ii
                e = in_cycle[ii % len(in_cycle)]
                ii += 1
                return e

            def out_eng():
                nonlocal oi
                e = out_cycle[oi % len(out_cycle)]
                oi += 1
                return e

            block_off = [sum(BLOCKS[:i]) for i in range(len(BLOCKS))]

            def load_block(b):
                # One tile per quarter: u-matmuls of a quarter start as soon
                # as it lands (tile deps are tile-granular). Block 0 goes
                # entirely on SP back-to-back for the earliest first chunk.
                # Exact-size tiles; a tile-pool tag must not mix sizes
                # (slots would overlap), so 128-row blocks get their own tag.
                # A partially-written max-size tile is no good either: the
                # 256B output runs double the modeled DMA latency.
                blk = BLOCKS[b]
                off = block_off[b] * KC
                tag = "xq" if blk == 256 else f"xq{blk}"
                tiles = []
                b0_cycle = [nc.sync, nc.gpsimd]
                for q in range(NQ):
                    t = xqp.tile([P, QK, blk], BF16, tag=tag, name=f"x{b}q{q}")
                    eng = b0_cycle[q % 2] if b == 0 else in_eng()
                    eng.dma_start(
                        t[:],
                        xt_d[:, off + q * QK * blk:off + (q + 1) * QK * blk]
                        .rearrange("p (kc r) -> p kc r", kc=QK),
                    )
                    tiles.append(t)
                return tiles

            def u_ops(b, xq_tiles):
                blk = BLOCKS[b]
                ut_ps = ps_u.tile([RANK, max(BLOCKS)], F32, tag="ut", name=f"u{b}")
                for q in range(NQ):
                    for k in range(QK):
                        kc = q * QK + k
                        yield lambda kc=kc, q=q, k=k: nc.tensor.matmul(
                            ut_ps[:, :blk],
                            a_sb[:, kc, :],
                            xq_tiles[q][:, k, :],
                            start=(kc == 0),
                            stop=(kc == KC - 1),
                        )
                ut_sb = usb.tile([RANK, max(BLOCKS)], BF16, tag="ut_sb",
                                 name=f"us{b}")
                if ut_eng == "vector":
                    yield lambda: nc.vector.tensor_copy(ut_sb[:, :blk],
                                                        ut_ps[:, :blk])
                else:
                    yield lambda: nc.scalar.copy(ut_sb[:, :blk], ut_ps[:, :blk])
                yield ("ut", ut_sb)

            drain_ci = 0

            def y_ops(b, ut_sb, out_chunk):
                # One [128,1024] PSUM tile at a time (2 bank-aligned matmuls,
                # one wide drain); out-DMA fires per out_chunk columns.
                nonlocal drain_ci
                blk = BLOCKS[b]
                row_base = block_off[b]
                for rt in range(blk // P):
                    row0 = row_base + rt * P
                    for j0 in range(0, D, out_chunk):
                        y_sb = ysb.tile([P, 2048], BF16, tag="yo",
                                        name=f"yo{b}r{rt}c{j0}")
                        for c in range(0, out_chunk, 1024):
                            y_ps = ps_y.tile([P, 1024], F32, tag="y_ps",
                                             name=f"y{b}r{rt}c{j0 + c}")
                            ops = []
                            for h in range(2):
                                j = j0 + c + h * 512
                                ops.append(lambda j=j, h=h, y_ps=y_ps, rt=rt:
                                           nc.tensor.matmul(
                                    y_ps[:, h * 512:(h + 1) * 512],
                                    ut_sb[:, rt * P:(rt + 1) * P],
                                    b_sb[:, j:j + 512],
                                ))
                            yield ("mm2", ops)
                            # Drain strategy: "split" halves each tile across
                            # ACT+DVE in parallel (best per-tile latency);
                            # "alt" alternates whole tiles between engines
                            # (best saturated throughput: 2 tiles in flight).
                            if drain_mode == "split":
                                yield lambda y_ps=y_ps, c=c, y_sb=y_sb: \
                                    nc.scalar.copy(y_sb[:, c:c + split],
                                                   y_ps[:, :split])
                                yield lambda y_ps=y_ps, c=c, y_sb=y_sb: \
                                    nc.vector.tensor_copy(
                                        y_sb[:, c + split:c + 1024],
                                        y_ps[:, split:])
                            elif drain_ci % 2 == 0:
                                yield lambda y_ps=y_ps, c=c, y_sb=y_sb: \
                                    nc.scalar.copy(y_sb[:, c:c + 1024], y_ps[:])
                            else:
                                yield lambda y_ps=y_ps, c=c, y_sb=y_sb: \
                                    nc.vector.tensor_copy(y_sb[:, c:c + 1024],
                                                          y_ps[:])
                            drain_ci += 1
                        yield lambda row0=row0, j0=j0, y_sb=y_sb: out_eng().dma_start(
                            y_d[row0:row0 + P, j0:j0 + out_chunk],
                            y_sb[:, :out_chunk],
                        )

            def run_ops(gen):
                """Drain an op generator fully; return its ut tile if any."""
                ut = None
                for item in gen:
                    if isinstance(item, tuple):
                        if item[0] == "ut":
                            ut = item[1]
                        else:
                            for f in item[1]:
                                f()
                    else:
                        item()
                return ut

            def interleave(ugen, ygen, lead=8):
                """Emit u-matmuls and y tiles interleaved so drain demand is
                spread across the block instead of bunched at its end."""
                uops = list(ugen)
                yitems = list(ygen)
                ut = None
                ui = yi = 0
                # lead u-ops first (ut copy of prev block needs ~0.8us).
                n_u = len(uops)
                n_y = len(yitems)
                while ui < n_u or yi < n_y:
                    take_u = min(lead if ui == 0 else max(1, (n_u - ui) // max(1, n_y - yi)), n_u - ui) if ui < n_u else 0
                    for _ in range(take_u):
                        item = uops[ui]
                        ui += 1
                        if isinstance(item, tuple):
                            ut = item[1]
                        else:
                            item()
                    if yi < n_y:
                        item = yitems[yi]
                        yi += 1
                        if isinstance(item, tuple):
                            for f in item[1]:
                                f()
                        else:
                            item()
                return ut

            # Prefetch block b+1's input DMAs BEFORE y(b-1)'s out DMAs hit
            # the same queues, so input never queues behind output.
            NB = len(BLOCKS)
            xq_next = load_block(0)
            ut_prev = None
            for b in range(NB):
                xq_cur = xq_next
                if b + 1 < NB:
                    xq_next = load_block(b + 1)
                if b == 0:
                    ut_prev = run_ops(u_ops(0, xq_cur))
                else:
                    # Final block: consume u-ops faster so the ut copy (whose
                    # consumer y(b) has no following u-phase for slack) isn't
                    # queued behind most of y(b-1)'s drains.
                    ld = lead_last if (b == NB - 1 and lead_last) else lead
                    ut_prev = interleave(
                        u_ops(b, xq_cur), y_ops(b - 1, ut_prev, out_chunk=2048),
                        lead=ld)
            run_ops(y_ops(NB - 1, ut_prev, out_chunk=1024))

    nc.compile()
    return nc


def get_nc(**build_kwargs):
    key = tuple(sorted(build_kwargs.items()))
    if key not in _NC_CACHE:
        _NC_CACHE[key] = build(**build_kwargs)
    return _NC_CACHE[key]


def _prep_in_maps(x, A, B):
    xf = np.asarray(x, dtype=np.float32).reshape(ROWS, D)
    af = np.asarray(A, dtype=np.float32) * np.float32(SCALE)
    a_prep = np.ascontiguousarray(
        af.reshape(KC, P, RANK).transpose(1, 0, 2)
    ).reshape(P, KC * RANK).astype(BF16_NP)
    bf = np.asarray(B, dtype=np.float32).astype(BF16_NP)
    out = []
    for c in range(N_CORES):
        shard = xf[c * R_CORE:(c + 1) * R_CORE]          # [1024, 4096]
        parts = []
        row0 = 0
        for blk in BLOCKS:
            sb = shard[row0:row0 + blk]                   # [blk, 4096]
            # [P, KC*blk] with xt[p, kc*blk + r] = sb[r, kc*128 + p]
            parts.append(
                np.ascontiguousarray(
                    sb.reshape(blk, KC, P).transpose(2, 1, 0)
                ).reshape(P, KC * blk)
            )
            row0 += blk
        xt = np.concatenate(parts, axis=1).astype(BF16_NP)
        out.append({"xt": xt, "A": a_prep, "B": bf})
    return out


def kernel(x, A, B, _nc=None, **run_kwargs):
    nc = _nc if _nc is not None else get_nc()
    in_maps = _prep_in_maps(x, A, B)
    try:
        res = run_bass_kernel_spmd(nc, in_maps, core_ids=list(range(N_CORES)),
                                   **run_kwargs)
    except Exception:
        if run_kwargs:
            raise
        # One retry: a first execution on a freshly-opened device has been
        # observed to fail transiently (NRT_EXEC_UNIT_UNRECOVERABLE); the
        # immediate rerun succeeds.
        res = run_bass_kernel_spmd(nc, in_maps, core_ids=list(range(N_CORES)))
    y = np.concatenate(
        [np.asarray(r["y"], dtype=np.float32) for r in res.results], axis=0
    )
    out = y.reshape(BATCH, SEQ, D)
    if run_kwargs:
        return out, res
    return out


# revision 55
# speedup vs baseline: 1.0140x; 1.0140x over previous
"""LoRA kernel for TRN2: y = (x @ A) @ B * scale, data-parallel over 8 cores.

Reference materializes W = (A@B)*scale [4096,4096] then x@W (~275 GFLOP).
Mathematically identical low-rank evaluation: u = x@(A*scale) [rows,8],
y = u@B — ~2 GFLOP, I/O + PE-streaming bound.

Per-core plan (rows sharded 8192/8 = 1024 rows/core, A/B replicated).

All matmul operands are bfloat16 (host converts; scale 2.0 is exact in
bf16): halves input DMA bytes vs f32 and keeps the PE at 1 col/cycle.
PSUM accumulation stays f32; y is drained PSUM->SBUF as bf16 and DMA'd
out as bf16 (host upcasts after the gather). Measured rel err ~5e-3
against the 2e-2 gate.

Cost-model facts this schedule is built around (CoreSim == grading
clock):
  - PE floor: (x elems + y elems)/128 = 65536 cyc @2.4GHz = 27.3us.
    P-state needs 3us of continuous PE busy — memset-fed warmups.
  - Every DMA->consumer edge costs ~2.2us fixed (HWDGE 630 + DGE 650 +
    sem 900); transfers run at 360 B/ns per queue, queues concurrent.
  - HWDGE (SP/ACT DMA) is a shared serializer ~630ns/instr; Pool DMAs
    (SWDGE) instead cost ~1us of Pool ENGINE each: keep instruction
    counts low, transfers big.
  - Tile deps are tile-granular: per-quarter x tiles, per-out-chunk y
    tiles.
  - Drains (PSUM [128,1024] f32 -> SBUF bf16): ACT 1038 / DVE 1192 ns.
    Interleaving y tiles into the NEXT block's u-matmuls on the PE
    spreads drain demand so the 2 drain engines never rate-lock the PE.

Row blocks [128, 256, 256, 256, 128]: small first block starts the PE
~0.8us earlier; small last block shortens the bare final y-phase tail.

Engine assignment:
  SP   : block-0 input quarters + half of remaining input + out DMAs
  Pool : other input quarters + out DMAs (SWDGE, private dead time)
  ACT  : A/B consts at t=0, half the y drains, a few out DMAs
  DVE  : memset warmup src, ut copies, other half of drains
  PE   : warmups; u(0); then u(b) interleaved with y(b-1); final y last

PSUM: ut double-buffer (2KB) + 3 x [128,1024] f32 y tiles (12KB)
= 14KB of 16KB.
"""

import os

import numpy as np
import ml_dtypes

os.environ.setdefault("MYCRO_LOCAL_CACHE", "1")

import concourse.bacc as bacc
import concourse.mybir as mybir
import concourse.tile as tile
from concourse.bass_utils import run_bass_kernel_spmd

F32 = mybir.dt.float32
BF16 = mybir.dt.bfloat16
BF16_NP = ml_dtypes.bfloat16

N_CORES = 8
BATCH, SEQ, D = 4, 2048, 4096
RANK = 8
SCALE = 16 / 8
ROWS = BATCH * SEQ            # 8192
R_CORE = ROWS // N_CORES      # 1024 rows per core
P = 128                       # partitions
KC = D // P                   # 32 feature chunks
NQ = 4                        # x-input quarters per block
QK = KC // NQ                 # 8 kc per quarter
BLOCKS = [128, 256, 256, 128, 128, 128]
assert sum(BLOCKS) == R_CORE

_NC_CACHE = {}


def build(warmup=5, lead=8, lead_last=4, ysb_bufs=5, split=536,
          drain_mode="alt", ut_eng="scalar", memset=True,
          a_eng="gpsimd", b_eng="scalar", n_last=1):
    nc = bacc.Bacc("TRN2", target_bir_lowering=False, debug=False)

    # Host packs x per-core as [P, R_CORE*KC] bf16 with
    # x row-block rb at col offset off*KC:  [p, (off+r)*... ] — see
    # _prep_in_maps: for each block, layout [P, KC*blk] with
    # xt[p, kc*blk + r] = x_shard[row0 + r, kc*128 + p].
    xt_d = nc.dram_tensor("xt", [P, R_CORE * KC], BF16, kind="ExternalInput")
    a_d = nc.dram_tensor("A", [P, KC * RANK], BF16, kind="ExternalInput")
    b_d = nc.dram_tensor("B", [RANK, D], BF16, kind="ExternalInput")
    y_d = nc.dram_tensor("y", [R_CORE, D], BF16, kind="ExternalOutput")

    with tile.TileContext(nc) as tc:
        with (
            tc.tile_pool(name="const", bufs=1) as cpool,
            tc.tile_pool(name="xq", bufs=8) as xqp,
            tc.tile_pool(name="usb", bufs=2) as usb,
            tc.tile_pool(name="ysb", bufs=ysb_bufs) as ysb,
            tc.tile_pool(name="ps_u", bufs=2, space="PSUM") as ps_u,
            tc.tile_pool(name="ps_y", bufs=(6 if drain_mode == "fine" else 3),
                         space="PSUM") as ps_y,
        ):
            # A gates the very first u-matmul: issue it at t=0.
            engs = {"sync": nc.sync, "gpsimd": nc.gpsimd, "scalar": nc.scalar}
            a_sb = cpool.tile([P, KC, RANK], BF16)
            engs[a_eng].dma_start(
                a_sb[:], a_d[:, :].rearrange("p (kc r) -> p kc r", kc=KC)
            )
            # B in 4 chunks (a single [8,4096] DMA costs ~3.2us in the model).
            b_sb = cpool.tile([RANK, D], BF16)
            for i in range(4):
                engs[b_eng].dma_start(b_sb[:, i * 1024:(i + 1) * 1024],
                                      b_d[:, i * 1024:(i + 1) * 1024])

            # Warmups ramp the PE p-state while input DMA streams; fed by a
            # small DVE memset so they don't wait on any DMA. Short 128-col
            # matmuls: pe_busy_start lands earliest, count bridges the gap
            # until real input arrives.
            w_src = cpool.tile([P, P], BF16)
            if warmup and memset:
                nc.vector.memset(w_src[:], 0.0)
            yp_cols = 512 if drain_mode == "fine" else 1024
            for w in range(warmup):
                w_ps = ps_y.tile([P, yp_cols], F32, tag="y_ps", name=f"w{w}")
                nc.tensor.matmul(w_ps[:, :P], w_src[:], w_src[:])

            # Out-DMAs NEVER go on ACT/DVE: a DMA whose sem-wait is pending
            # holds that engine's SEQ, stalling the drain queue behind it.
            in_cycle = [nc.sync, nc.gpsimd]
            out_cycle = [nc.gpsimd, nc.sync]
            ii = oi = 0

            def in_eng():
                nonlocal ii
                e = in_cycle[ii % len(in_cycle)]
                ii += 1
                return e

            def out_eng():
                nonlocal oi
                e = out_cycle[oi % len(out_cycle)]
                oi += 1
                return e

            block_off = [sum(BLOCKS[:i]) for i in range(len(BLOCKS))]

            def load_block(b):
                # One tile per quarter: u-matmuls of a quarter start as soon
                # as it lands (tile deps are tile-granular). Block 0 goes
                # entirely on SP back-to-back for the earliest first chunk.
                # Exact-size tiles; a tile-pool tag must not mix sizes
                # (slots would overlap), so 128-row blocks get their own tag.
                # A partially-written max-size tile is no good either: the
                # 256B output runs double the modeled DMA latency.
                blk = BLOCKS[b]
                off = block_off[b] * KC
                tag = "xq" if blk == 256 else f"xq{blk}"
                tiles = []
                b0_cycle = [nc.sync, nc.gpsimd]
                for q in range(NQ):
                    t = xqp.tile([P, QK, blk], BF16, tag=tag, name=f"x{b}q{q}")
                    eng = b0_cycle[q % 2] if b == 0 else in_eng()
                    eng.dma_start(
                        t[:],
                        xt_d[:, off + q * QK * blk:off + (q + 1) * QK * blk]
                        .rearrange("p (kc r) -> p kc r", kc=QK),
                    )
                    tiles.append(t)
                return tiles

            def u_ops(b, xq_tiles):
                blk = BLOCKS[b]
                ut_ps = ps_u.tile([RANK, max(BLOCKS)], F32, tag="ut", name=f"u{b}")
                for q in range(NQ):
                    for k in range(QK):
                        kc = q * QK + k
                        yield lambda kc=kc, q=q, k=k: nc.tensor.matmul(
                            ut_ps[:, :blk],
                            a_sb[:, kc, :],
                            xq_tiles[q][:, k, :],
                            start=(kc == 0),
                            stop=(kc == KC - 1),
                        )
                ut_sb = usb.tile([RANK, max(BLOCKS)], BF16, tag="ut_sb",
                                 name=f"us{b}")
                def ut_copy():
                    # "auto": copy on whichever drain engine was queued least
                    # recently (the copy gates the next y-phase start).
                    eng = ut_eng
                    if eng == "auto":
                        eng = "vector" if drain_ci % 2 == 1 else "scalar"
                    if eng == "vector":
                        nc.vector.tensor_copy(ut_sb[:, :blk], ut_ps[:, :blk])
                    else:
                        nc.scalar.copy(ut_sb[:, :blk], ut_ps[:, :blk])
                yield ut_copy
                yield ("ut", ut_sb)

            drain_ci = 0

            def y_ops_tail(b, ut_sb):
                # Final block: per-tile split drains (ACT+DVE in parallel,
                # ~658ns latency) and 512-wide DMAs on both queues — the
                # drain/DMA chain after the last matmul is what the program
                # ends on.
                blk = BLOCKS[b]
                row_base = block_off[b]
                for rt in range(blk // P):
                    row0 = row_base + rt * P
                    for j0 in range(0, D, 1024):
                        y_ps = ps_y.tile([P, 1024], F32, tag="y_ps",
                                         name=f"y{b}r{rt}c{j0}")
                        ops = []
                        for h in range(2):
                            j = j0 + h * 512
                            ops.append(lambda j=j, h=h, y_ps=y_ps, rt=rt:
                                       nc.tensor.matmul(
                                           y_ps[:, h * 512:(h + 1) * 512],
                                           ut_sb[:, rt * P:(rt + 1) * P],
                                           b_sb[:, j:j + 512]))
                        yield ("mm2", ops)
                        y_sb = ysb.tile([P, 2048], BF16, tag="yo",
                                        name=f"yo{b}r{rt}c{j0}")
                        yield lambda y_ps=y_ps, y_sb=y_sb: \
                            nc.scalar.copy(y_sb[:, :512], y_ps[:, :512])
                        yield lambda y_ps=y_ps, y_sb=y_sb: \
                            nc.vector.tensor_copy(y_sb[:, 512:1024],
                                                  y_ps[:, 512:])
                        for h in range(2):
                            yield lambda row0=row0, j0=j0, h=h, y_sb=y_sb: \
                                out_eng().dma_start(
                                    y_d[row0:row0 + P,
                                        j0 + h * 512:j0 + (h + 1) * 512],
                                    y_sb[:, h * 512:(h + 1) * 512])

            def y_ops(b, ut_sb, out_chunk, final=False, flip=False):
                # One [128,1024] PSUM tile at a time (2 bank-aligned matmuls,
                # one wide drain); out-DMA fires per out_chunk columns.
                nonlocal drain_ci
                blk = BLOCKS[b]
                row_base = block_off[b]
                if drain_mode == "fine":
                    # One [P,512] PSUM tile + one drain per matmul; 6 PSUM
                    # slots decouple the PE from the drain latency chain.
                    for rt in range(blk // P):
                        row0 = row_base + rt * P
                        for j0 in range(0, D, out_chunk):
                            y_sb = ysb.tile([P, 2048], BF16, tag="yo",
                                            name=f"yo{b}r{rt}c{j0}")
                            for c in range(0, out_chunk, 512):
                                j = j0 + c
                                y_ps = ps_y.tile([P, 512], F32, tag="y_ps",
                                                 name=f"y{b}r{rt}c{j}")
                                yield ("mm2", [lambda j=j, y_ps=y_ps, rt=rt:
                                               nc.tensor.matmul(
                                                   y_ps[:],
                                                   ut_sb[:, rt * P:(rt + 1) * P],
                                                   b_sb[:, j:j + 512])])
                                if drain_ci % 2 == 0:
                                    yield lambda y_ps=y_ps, c=c, y_sb=y_sb: \
                                        nc.scalar.copy(y_sb[:, c:c + 512],
                                                       y_ps[:])
                                else:
                                    yield lambda y_ps=y_ps, c=c, y_sb=y_sb: \
                                        nc.vector.tensor_copy(
                                            y_sb[:, c:c + 512], y_ps[:])
                                drain_ci += 1
                            yield lambda row0=row0, j0=j0, y_sb=y_sb: \
                                out_eng().dma_start(
                                    y_d[row0:row0 + P, j0:j0 + out_chunk],
                                    y_sb[:, :out_chunk])
                    return
                for rt in range(blk // P):
                    row0 = row_base + rt * P
                    for j0 in range(0, D, out_chunk):
                        y_sb = ysb.tile([P, 2048], BF16, tag="yo",
                                        name=f"yo{b}r{rt}c{j0}")
                        for c in range(0, out_chunk, 1024):
                            y_ps = ps_y.tile([P, 1024], F32, tag="y_ps",
                                             name=f"y{b}r{rt}c{j0 + c}")
                            ops = []
                            for h in range(2):
                                j = j0 + c + h * 512
                                ops.append(lambda j=j, h=h, y_ps=y_ps, rt=rt:
                                           nc.tensor.matmul(
                                    y_ps[:, h * 512:(h + 1) * 512],
                                    ut_sb[:, rt * P:(rt + 1) * P],
                                    b_sb[:, j:j + 512],
                                ))
                            yield ("mm2", ops)
                            # Drain strategy: "split" halves each tile across
                            # ACT+DVE in parallel (best per-tile latency);
                            # "alt" alternates whole tiles between engines
                            # (best saturated throughput: 2 tiles in flight).
                            if drain_mode == "split":
                                yield lambda y_ps=y_ps, c=c, y_sb=y_sb: \
                                    nc.scalar.copy(y_sb[:, c:c + split],
                                                   y_ps[:, :split])
                                yield lambda y_ps=y_ps, c=c, y_sb=y_sb: \
                                    nc.vector.tensor_copy(
                                        y_sb[:, c + split:c + 1024],
                                        y_ps[:, split:])
                            elif (drain_ci + (1 if (final and flip) else 0)) % 2 == 0:
                                # `flip` flips parity so the program's very
                                # last tile drains on ACT (1038 < DVE 1192).
                                yield lambda y_ps=y_ps, c=c, y_sb=y_sb: \
                                    nc.scalar.copy(y_sb[:, c:c + 1024], y_ps[:])
                            else:
                                yield lambda y_ps=y_ps, c=c, y_sb=y_sb: \
                                    nc.vector.tensor_copy(y_sb[:, c:c + 1024],
                                                          y_ps[:])
                            drain_ci += 1
                        is_last = (final and rt == blk // P - 1
                                   and j0 + out_chunk >= D)
                        if is_last:
                            # Split the program's final out-DMA across both
                            # queues: it is on the end-to-end critical path.
                            for h in range(2):
                                yield lambda row0=row0, j0=j0, h=h, y_sb=y_sb: \
                                    out_eng().dma_start(
                                        y_d[row0:row0 + P,
                                            j0 + h * 512:j0 + (h + 1) * 512],
                                        y_sb[:, h * 512:(h + 1) * 512])
                        else:
                            yield lambda row0=row0, j0=j0, y_sb=y_sb: \
                                out_eng().dma_start(
                                    y_d[row0:row0 + P, j0:j0 + out_chunk],
                                    y_sb[:, :out_chunk],
                                )

            def run_ops(gen):
                """Drain an op generator fully; return its ut tile if any."""
                ut = None
                for item in gen:
                    if isinstance(item, tuple):
                        if item[0] == "ut":
                            ut = item[1]
                        else:
                            for f in item[1]:
                                f()
                    else:
                        item()
                return ut

            def interleave(ugen, ygen, lead=8):
                """Emit u-matmuls and y tiles interleaved so drain demand is
                spread across the block instead of bunched at its end."""
                uops = list(ugen)
                yitems = list(ygen)
                ut = None
                ui = yi = 0
                # lead u-ops first (ut copy of prev block needs ~0.8us).
                n_u = len(uops)
                n_y = len(yitems)
                while ui < n_u or yi < n_y:
                    take_u = min(lead if ui == 0 else max(1, (n_u - ui) // max(1, n_y - yi)), n_u - ui) if ui < n_u else 0
                    for _ in range(take_u):
                        item = uops[ui]
                        ui += 1
                        if isinstance(item, tuple):
                            ut = item[1]
                        else:
                            item()
                    if yi < n_y:
                        item = yitems[yi]
                        yi += 1
                        if isinstance(item, tuple):
                            for f in item[1]:
                                f()
                        else:
                            item()
                return ut

            # Prefetch block b+1's input DMAs BEFORE y(b-1)'s out DMAs hit
            # the same queues, so input never queues behind output.
            NB = len(BLOCKS)
            xq_next = load_block(0)
            ut_prev = None
            for b in range(NB):
                xq_cur = xq_next
                if b + 1 < NB:
                    xq_next = load_block(b + 1)
                if b == 0:
                    ut_prev = run_ops(u_ops(0, xq_cur))
                else:
                    # Final block: consume u-ops faster so the ut copy (whose
                    # consumer y(b) has no following u-phase for slack) isn't
                    # queued behind most of y(b-1)'s drains.
                    ld = lead_last if (b >= NB - n_last and lead_last) else lead
                    ut_prev = interleave(
                        u_ops(b, xq_cur), y_ops(b - 1, ut_prev, out_chunk=2048),
                        lead=ld)
            run_ops(y_ops(NB - 1, ut_prev, out_chunk=1024))

    nc.compile()
    return nc


def get_nc(**build_kwargs):
    key = tuple(sorted(build_kwargs.items()))
    if key not in _NC_CACHE:
        _NC_CACHE[key] = build(**build_kwargs)
    return _NC_CACHE[key]


def _prep_in_maps(x, A, B):
    xf = np.asarray(x, dtype=np.float32).reshape(ROWS, D)
    af = np.asarray(A, dtype=np.float32) * np.float32(SCALE)
    a_prep = np.ascontiguousarray(
        af.reshape(KC, P, RANK).transpose(1, 0, 2)
    ).reshape(P, KC * RANK).astype(BF16_NP)
    bf = np.asarray(B, dtype=np.float32).astype(BF16_NP)
    out = []
    for c in range(N_CORES):
        shard = xf[c * R_CORE:(c + 1) * R_CORE]          # [1024, 4096]
        parts = []
        row0 = 0
        for blk in BLOCKS:
            sb = shard[row0:row0 + blk]                   # [blk, 4096]
            # [P, KC*blk] with xt[p, kc*blk + r] = sb[r, kc*128 + p]
            parts.append(
                np.ascontiguousarray(
                    sb.reshape(blk, KC, P).transpose(2, 1, 0)
                ).reshape(P, KC * blk)
            )
            row0 += blk
        xt = np.concatenate(parts, axis=1).astype(BF16_NP)
        out.append({"xt": xt, "A": a_prep, "B": bf})
    return out


def kernel(x, A, B, _nc=None, **run_kwargs):
    nc = _nc if _nc is not None else get_nc()
    in_maps = _prep_in_maps(x, A, B)
    try:
        res = run_bass_kernel_spmd(nc, in_maps, core_ids=list(range(N_CORES)),
                                   **run_kwargs)
    except Exception:
        if run_kwargs:
            raise
        # One retry: a first execution on a freshly-opened device has been
        # observed to fail transiently (NRT_EXEC_UNIT_UNRECOVERABLE); the
        # immediate rerun succeeds.
        res = run_bass_kernel_spmd(nc, in_maps, core_ids=list(range(N_CORES)))
    y = np.concatenate(
        [np.asarray(r["y"], dtype=np.float32) for r in res.results], axis=0
    )
    out = y.reshape(BATCH, SEQ, D)
    if run_kwargs:
        return out, res
    return out
